# revision 8
# baseline (speedup 1.0000x reference)
"""Trainium2 Bass kernel for a 2-layer GRU controller step (batch=1).

Model (PyTorch GRU-cell semantics, gates packed [r, z, n]):
    e  = emb[x]                                  [1, 512]
    h0 = GRUCell(e,  h_in[0]; w_ih0, w_hh0, b_ih0, b_hh0)   H=2048
    h1 = GRUCell(h0, h_in[1]; w_ih1, w_hh1, b_ih1, b_hh1)
    probs = softmax(h1 @ w_head.T + b_head)      [1, 16]
    h_out = stack([h0, h1])                      [2, 1, 2048]

Distribution (8 NeuronCores, tensor-parallel on the gate dimension):
  Each core owns a 256-row slice of every gate (r/z/n) of both layers =
  768 rows of each packed weight matrix, host-pre-transposed to a
  K-major blob [C, 768].  Matvecs run with the *weights as the moving
  operand*: lhsT = x k-chunk [128, 1] (stationary, 1-column weight
  load), rhs = W_T k-slab [128, 768] streaming at one column/cycle, so
  the PE cost is ~elements/128 cycles with no per-tile LDWEIGHTS.
  Outputs accumulate as [1, 768] on partition 0: PSUM bank A [1,512]
  carries i_rz+h_rz (gi and gh share the accumulation), bank B packs
  i_n | h_n.  Gate math runs on partition 0; the resulting h shard
  [1,256] DMAs contiguously into an AllGather across the 8 cores.
  Layer-1 consumes the gathered h0 as new stationary chunks; the head
  + softmax is computed redundantly on every core after the h1
  AllGather.  Weights move in 7 large DMAs (1-1.6 MB each) to keep the
  Sync dispatch queue and DMA-completion semaphore lanes free.

Memory roofline: ~163 MB f32 of weights / 8 cores; bf16 halves it
(~10.3 MB/core at ~360 GB/s -> ~29 us DMA floor).
"""

import os

import numpy as np

H = 2048
E = 512
LEN_ACTION = 64
NV = 16
NCORES = 8
SH = H // NCORES          # 256 hidden units per core
R = 3 * SH                # 768 gate rows per core per matrix
KH = H // 128             # 16 k-chunks for hidden-sized contractions
KE = E // 128             # 4 k-chunks for embed-sized contraction

# Weight dtype: "f32" or "bf16" (bf16 halves HBM traffic; ~2e-3 rel err)
WDT = os.environ.get("BASS_GRU_WDT", "bf16")

_CACHE = {}
LAST_RESULTS = None


def _build(wdt_name: str):
    import concourse.bacc as bacc
    import concourse.mybir as mybir
    import concourse.tile as tile

    F32 = mybir.dt.float32
    I32 = mybir.dt.int32
    WD = F32 if wdt_name == "f32" else mybir.dt.bfloat16

    nc = bacc.Bacc("TRN2", target_bir_lowering=False, debug=False,
                   num_devices=NCORES)

    x_idx = nc.dram_tensor("x_idx", [1, 1], I32, kind="ExternalInput")
    emb = nc.dram_tensor("emb", [LEN_ACTION, E], WD, kind="ExternalInput")
    hin_rhs = nc.dram_tensor("hin_rhs", [128, 2 * KH], WD,
                             kind="ExternalInput")
    hp = nc.dram_tensor("hp", [1, 2 * SH], F32, kind="ExternalInput")
    biases = nc.dram_tensor("biases", [1, 2 * 4 * SH], F32,
                            kind="ExternalInput")
    bhead = nc.dram_tensor("bhead", [1, NV], F32, kind="ExternalInput")
    wih0T = nc.dram_tensor("wih0T", [E, R], WD, kind="ExternalInput")
    whh0T = nc.dram_tensor("whh0T", [H, R], WD, kind="ExternalInput")
    wih1T = nc.dram_tensor("wih1T", [H, R], WD, kind="ExternalInput")
    whh1T = nc.dram_tensor("whh1T", [H, R], WD, kind="ExternalInput")
    wheadT = nc.dram_tensor("wheadT", [128, KH * NV], WD,
                            kind="ExternalInput")
    probs_o = nc.dram_tensor("probs", [1, NV], F32, kind="ExternalOutput")
    hout_o = nc.dram_tensor("h_out", [2, H], F32, kind="ExternalOutput")

    with tile.TileContext(nc) as tc:
        with (
            tc.tile_pool(name="wp", bufs=1) as wp,
            tc.tile_pool(name="sb", bufs=1) as sb,
            tc.tile_pool(name="ps", bufs=1, space="PSUM") as ps,
            tc.tile_pool(name="dram", bufs=1, space="DRAM") as dram,
        ):
            # ---------- small input loads ----------
            xs = sb.tile([1, 1], I32, tag="xs")
            nc.sync.dma_start(xs[:], x_idx[:, :])
            embsb = sb.tile([LEN_ACTION, E], WD, tag="embsb")
            nc.sync.dma_start(embsb[:], emb[:, :])
            hin_sb = sb.tile([128, 2 * KH], WD, tag="hin_sb")
            nc.sync.dma_start(hin_sb[:], hin_rhs[:, :])
            hp_sb = sb.tile([1, 2 * SH], F32, tag="hp_sb")
            nc.sync.dma_start(hp_sb[:], hp[:, :])
            b_sb = sb.tile([1, 2 * 4 * SH], F32, tag="b_sb")
            nc.sync.dma_start(b_sb[:], biases[:, :])
            whead_sb = sb.tile([128, KH * NV], WD, tag="whead_sb")
            nc.sync.dma_start(whead_sb[:], wheadT[:, :])
            bh_sb = sb.tile([1, NV], F32, tag="bh_sb")
            nc.sync.dma_start(bh_sb[:], bhead[:, :])

            # ---------- weight loads: 7 big DMAs ----------
            def wload(wdram, nk, name, halves):
                tiles = []
                per = nk // halves
                for hhf in range(halves):
                    t = wp.tile([128, per * R], WD, tag=f"{name}_{hhf}")
                    src = wdram.ap()[hhf * per * 128:(hhf + 1) * per * 128, :]
                    nc.sync.dma_start(
                        t[:].rearrange("p (kb m) -> p kb m", kb=per),
                        src.rearrange("(kb p) m -> p kb m", p=128))
                    tiles.append(t)

                def slab(k):          # [128, R] k-slab view
                    hhf, kb = divmod(k, per)
                    return tiles[hhf][:, kb * R:(kb + 1) * R]
                return slab

            wih0 = wload(wih0T, KE, "wih0", 1)
            whh0 = wload(whh0T, KH, "whh0", 2)
            whh1 = wload(whh1T, KH, "whh1", 2)
            wih1 = wload(wih1T, KH, "wih1", 2)

            # ---------- e = emb[x] via one-hot matmul ----------
            xf = sb.tile([1, 1], F32, tag="xf")
            nc.vector.tensor_copy(xf[:], xs[:])
            ones = sb.tile([1, 128], F32, tag="ones")
            nc.vector.memset(ones[:], 1.0)
            pse = ps.tile([128, KE + 1], F32, tag="pse")
            nc.tensor.matmul(pse[:, KE:KE + 1], ones[:], xf[:],
                             start=True, stop=False, skip_group_check=True)
            iot = sb.tile([128, 1], F32, tag="iot")
            nc.gpsimd.iota(iot[:], [[0, 1]], channel_multiplier=1,
                           allow_small_or_imprecise_dtypes=True)
            oneh = sb.tile([128, 1], WD, tag="oneh")
            nc.vector.tensor_tensor(oneh[:], iot[:], pse[:, KE:KE + 1],
                                    mybir.AluOpType.is_equal)
            for c in range(KE):
                nc.tensor.matmul(pse[:, c:c + 1],
                                 embsb[0:LEN_ACTION, c * 128:(c + 1) * 128],
                                 oneh[0:LEN_ACTION, 0:1],
                                 start=False, stop=(c == KE - 1),
                                 skip_group_check=True)
            e_sb = sb.tile([128, KE], WD, tag="e_sb")
            nc.vector.tensor_copy(e_sb[:], pse[:, 0:KE])

            # ---------- matvec block: weights stream as rhs ----------
            def mm_block(pA, pBC, slab_fn, nk, x_fn, n_off, firstA, firstBC,
                         lastA, lastBC):
                """psum A [1,512] += slab[:, 0:512].T @ x  (rz, gi+gh fused)
                psum BC[1, n_off:n_off+256] (+)= slab[:, 512:768].T @ x."""
                for k in range(nk):
                    sl = slab_fn(k)
                    xk = x_fn(k)
                    nc.tensor.matmul(pA[0:1, :], xk, sl[:, 0:2 * SH],
                                     start=firstA and k == 0,
                                     stop=lastA and k == nk - 1,
                                     skip_group_check=True)
                    nc.tensor.matmul(pBC[0:1, n_off:n_off + SH], xk,
                                     sl[:, 2 * SH:3 * SH],
                                     start=firstBC and k == 0,
                                     stop=lastBC and k == nk - 1,
                                     skip_group_check=True)

            def gates(pA, pBC, l):
                """GRU gate math on partition 0.
                pA [1,512] = i_rz+h_rz; pBC [1,512] = i_n | h_n."""
                bo = l * 4 * SH
                rz_b = sb.tile([1, 2 * SH], F32, tag=f"rzb{l}")
                nc.vector.tensor_add(rz_b[:], pA[0:1, :],
                                     b_sb[0:1, bo:bo + 2 * SH])
                rz = sb.tile([1, 2 * SH], F32, tag=f"rz{l}")
                nc.scalar.activation(rz[:], rz_b[:],
                                     mybir.ActivationFunctionType.Sigmoid)
                in_b = sb.tile([1, SH], F32, tag=f"inb{l}")
                nc.vector.tensor_add(in_b[:], pBC[0:1, 0:SH],
                                     b_sb[0:1, bo + 2 * SH:bo + 3 * SH])
                hn_b = sb.tile([1, SH], F32, tag=f"hnb{l}")
                nc.vector.tensor_add(hn_b[:], pBC[0:1, SH:2 * SH],
                                     b_sb[0:1, bo + 3 * SH:bo + 4 * SH])
                rhn = sb.tile([1, SH], F32, tag=f"rhn{l}")
                nc.vector.tensor_mul(rhn[:], rz[0:1, 0:SH], hn_b[:])
                npre = sb.tile([1, SH], F32, tag=f"npre{l}")
                nc.vector.tensor_add(npre[:], in_b[:], rhn[:])
                n = sb.tile([1, SH], F32, tag=f"n{l}")
                nc.scalar.activation(n[:], npre[:],
                                     mybir.ActivationFunctionType.Tanh)
                d = sb.tile([1, SH], F32, tag=f"d{l}")
                nc.vector.tensor_sub(d[:], hp_sb[0:1, l * SH:(l + 1) * SH],
                                     n[:])
                zd = sb.tile([1, SH], F32, tag=f"zd{l}")
                nc.vector.tensor_mul(zd[:], rz[0:1, SH:2 * SH], d[:])
                h = sb.tile([1, SH], F32, tag=f"h{l}")
                nc.vector.tensor_add(h[:], n[:], zd[:])
                return h

            def allgather(h_tile, l):
                ag_in = dram.tile([SH], mybir.dt.float32, tag=f"agi{l}")
                ag_out = dram.tile([H], mybir.dt.float32, tag=f"ago{l}")
                nc.sync.dma_start(ag_in[:].unsqueeze(0), h_tile[0:1, :])
                nc.gpsimd.collective_compute(
                    "AllGather",
                    mybir.AluOpType.bypass,
                    replica_groups=[list(range(NCORES))],
                    ins=[ag_in[:].opt()],
                    outs=[ag_out[:].opt()],
                )
                # chunked [128, 16] readback (lhsT x-chunks for next use)
                hf = sb.tile([128, KH], F32, tag=f"hf{l}")
                nc.sync.dma_start(hf[:],
                                  ag_out.rearrange("(c p) -> p c", p=128))
                # full-state output row l
                nc.sync.dma_start(
                    hout_o.ap()[l, :].rearrange("(c p) -> p c", p=128),
                    hf[:])
                if WD is F32:
                    return hf
                hfw = sb.tile([128, KH], WD, tag=f"hfw{l}")
                nc.vector.tensor_copy(hfw[:], hf[:])
                return hfw

            # ---------- layer 0 ----------
            pA0 = ps.tile([1, 2 * SH], F32, tag="pA0")
            pBC0 = ps.tile([1, 2 * SH], F32, tag="pBC0")
            mm_block(pA0, pBC0, wih0, KE, lambda k: e_sb[:, k:k + 1],
                     n_off=0, firstA=True, firstBC=True,
                     lastA=False, lastBC=True)
            mm_block(pA0, pBC0, whh0, KH, lambda k: hin_sb[:, k:k + 1],
                     n_off=SH, firstA=False, firstBC=False,
                     lastA=True, lastBC=True)
            h0 = gates(pA0, pBC0, 0)
            h0f = allgather(h0, 0)

            # ---------- layer 1 (gh first: its rhs is known from t=0) ----
            pA1 = ps.tile([1, 2 * SH], F32, tag="pA1")
            pBC1 = ps.tile([1, 2 * SH], F32, tag="pBC1")
            mm_block(pA1, pBC1, whh1, KH,
                     lambda k: hin_sb[:, KH + k:KH + k + 1],
                     n_off=SH, firstA=True, firstBC=True,
                     lastA=False, lastBC=True)
            mm_block(pA1, pBC1, wih1, KH, lambda k: h0f[:, k:k + 1],
                     n_off=0, firstA=False, firstBC=False,
                     lastA=True, lastBC=True)
            h1 = gates(pA1, pBC1, 1)
            h1f = allgather(h1, 1)

            # ---------- head + softmax (redundant on every core) --------
            psh = ps.tile([1, NV], F32, tag="psh")
            for c in range(KH):
                nc.tensor.matmul(psh[0:1, :], h1f[:, c:c + 1],
                                 whead_sb[:, c * NV:(c + 1) * NV],
                                 start=(c == 0), stop=(c == KH - 1))
            logit = sb.tile([1, NV], F32, tag="logit")
            nc.vector.tensor_add(logit[:], psh[0:1, :], bh_sb[:])
            lmax = sb.tile([1, 1], F32, tag="lmax")
            nc.vector.tensor_reduce(lmax[:], logit[:],
                                    axis=mybir.AxisListType.X,
                                    op=mybir.AluOpType.max)
            shift = sb.tile([1, NV], F32, tag="shift")
            nc.vector.tensor_scalar(shift[:], logit[:], lmax[0:1, 0:1], None,
                                    mybir.AluOpType.subtract)
            ex = sb.tile([1, NV], F32, tag="ex")
            nc.scalar.activation(ex[:], shift[:],
                                 mybir.ActivationFunctionType.Exp)
            ssum = sb.tile([1, 1], F32, tag="ssum")
            nc.vector.tensor_reduce(ssum[:], ex[:],
                                    axis=mybir.AxisListType.X,
                                    op=mybir.AluOpType.add)
            rs = sb.tile([1, 1], F32, tag="rs")
            nc.vector.reciprocal(rs[:], ssum[:])
            prb = sb.tile([1, NV], F32, tag="prb")
            nc.vector.tensor_scalar(prb[:], ex[:], rs[0:1, 0:1], None,
                                    mybir.AluOpType.mult)
            nc.sync.dma_start(probs_o[:, :], prb[:])

    nc.compile()
    return nc


def _chunk(v):
    """[n*128] -> [128, n] chunked layout: out[p, c] = v[c*128 + p]."""
    return np.ascontiguousarray(v.reshape(-1, 128).T)


def _prep_inputs(x, h_in, emb, w_ih0, w_hh0, b_ih0, b_hh0,
                 w_ih1, w_hh1, b_ih1, b_hh1, w_head, b_head, np_wdt):
    """Build the per-core input maps (shard + transpose + cast)."""
    f32 = np.float32
    x_i = np.asarray(x).astype(np.int32).reshape(1, 1)
    h_in = np.asarray(h_in, f32)
    emb = np.ascontiguousarray(np.asarray(emb)).astype(np_wdt)
    w = {0: (np.asarray(w_ih0, f32), np.asarray(w_hh0, f32)),
         1: (np.asarray(w_ih1, f32), np.asarray(w_hh1, f32))}
    b = {0: (np.asarray(b_ih0, f32), np.asarray(b_hh0, f32)),
         1: (np.asarray(b_ih1, f32), np.asarray(b_hh1, f32))}
    w_head = np.asarray(w_head, f32)
    b_head = np.asarray(b_head, f32).reshape(1, NV)

    hin_rhs = np.concatenate([_chunk(h_in[0, 0]), _chunk(h_in[1, 0])],
                             axis=1).astype(np_wdt)
    # wheadT[p, c*16+j] = w_head[j, c*128+p]
    wheadT = np.ascontiguousarray(
        w_head.T.reshape(KH, 128, NV).transpose(1, 0, 2).reshape(128, KH * NV)
    ).astype(np_wdt)

    in_maps = []
    for k in range(NCORES):
        sl = slice(k * SH, (k + 1) * SH)

        def rows(mat):
            return np.concatenate([mat[g * H:(g + 1) * H][sl]
                                   for g in range(3)], axis=0)

        core = {
            "x_idx": x_i, "emb": emb, "hin_rhs": hin_rhs, "wheadT": wheadT,
            "bhead": b_head,
            "hp": np.concatenate([h_in[0, 0, sl],
                                  h_in[1, 0, sl]]).reshape(1, 2 * SH),
        }
        for l in (0, 1):
            wih, whh = w[l]
            core[f"wih{l}T"] = np.ascontiguousarray(rows(wih).T).astype(np_wdt)
            core[f"whh{l}T"] = np.ascontiguousarray(rows(whh).T).astype(np_wdt)
        blobs = []
        for l in (0, 1):
            bih, bhh = b[l]
            brz = (bih + bhh)
            blobs += [np.concatenate([brz[g * H:(g + 1) * H][sl]
                                      for g in range(2)]),
                      bih[2 * H:][sl], bhh[2 * H:][sl]]
        core["biases"] = np.concatenate(blobs).reshape(1, -1).astype(f32)
        in_maps.append(core)
    return in_maps


def kernel(x, h_in, emb, w_ih0, w_hh0, b_ih0, b_hh0,
           w_ih1, w_hh1, b_ih1, b_hh1, w_head, b_head):
    global LAST_RESULTS
    import ml_dtypes
    from concourse.bass_utils import run_bass_kernel_spmd

    wdt = WDT
    np_wdt = np.float32 if wdt == "f32" else ml_dtypes.bfloat16
    trace = bool(int(os.environ.get("BASS_KERNEL_TRACE", "0")))

    key = (wdt,)
    if key not in _CACHE:
        _CACHE[key] = _build(wdt)
    nc = _CACHE[key]

    in_maps = _prep_inputs(x, h_in, emb, w_ih0, w_hh0, b_ih0, b_hh0,
                           w_ih1, w_hh1, b_ih1, b_hh1, w_head, b_head,
                           np_wdt)
    res = run_bass_kernel_spmd(nc, in_maps, core_ids=list(range(NCORES)),
                               trace=trace)
    LAST_RESULTS = res
    probs = np.asarray(res.results[0]["probs"], np.float32)
    h_out = np.asarray(res.results[0]["h_out"],
                       np.float32).reshape(2, 1, H)
    return probs, h_out


# revision 16
# speedup vs baseline: 1.1014x; 1.1014x over previous
"""Trainium2 Bass kernel for a 2-layer GRU controller step (batch=1).

Model (PyTorch GRU-cell semantics, gates packed [r, z, n]):
    e  = emb[x]                                  [1, 512]
    h0 = GRUCell(e,  h_in[0]; w_ih0, w_hh0, b_ih0, b_hh0)   H=2048
    h1 = GRUCell(h0, h_in[1]; w_ih1, w_hh1, b_ih1, b_hh1)
    probs = softmax(h1 @ w_head.T + b_head)      [1, 16]
    h_out = stack([h0, h1])                      [2, 1, 2048]

Distribution (8 NeuronCores, tensor-parallel on the gate dimension):
  Each core owns a 256-row slice of every gate (r/z/n) of both layers =
  768 rows of each packed weight matrix, host-pre-transposed to a
  K-major blob [C, 768].  Matvecs run with the *weights as the moving
  operand*: lhsT = x k-chunk [128, 1] (stationary, 1-column weight
  load), rhs = W_T k-slab [128, 768] streaming at one column/cycle, so
  the PE cost is ~elements/128 cycles with no per-tile LDWEIGHTS.
  Outputs accumulate as [1, 768] on partition 0: PSUM bank A [1,512]
  carries i_rz+h_rz (gi and gh share the accumulation), bank B packs
  i_n | h_n.  Gate math runs on partition 0; the resulting h shard
  [1,256] DMAs contiguously into an AllGather across the 8 cores.
  Layer-1 consumes the gathered h0 as new stationary chunks; the head
  + softmax is computed redundantly on every core after the h1
  AllGather.  Weights move in 7 large DMAs (1-1.6 MB each) to keep the
  Sync dispatch queue and DMA-completion semaphore lanes free.

Memory roofline: ~163 MB f32 of weights / 8 cores; bf16 halves it
(~10.3 MB/core at ~360 GB/s -> ~29 us DMA floor).
"""

import os

import numpy as np

H = 2048
E = 512
LEN_ACTION = 64
NV = 16
NCORES = 8
SH = H // NCORES          # 256 hidden units per core
R = 3 * SH                # 768 gate rows per core per matrix
KH = H // 128             # 16 k-chunks for hidden-sized contractions
KE = E // 128             # 4 k-chunks for embed-sized contraction

# Weight dtype: "f32" or "bf16" (bf16 halves HBM traffic; ~2e-3 rel err)
WDT = os.environ.get("BASS_GRU_WDT", "bf16")

_CACHE = {}
LAST_RESULTS = None


def _build(wdt_name: str):
    import concourse.bacc as bacc
    import concourse.mybir as mybir
    import concourse.tile as tile

    F32 = mybir.dt.float32
    I32 = mybir.dt.int32
    WD = F32 if wdt_name == "f32" else mybir.dt.bfloat16

    nc = bacc.Bacc("TRN2", target_bir_lowering=False, debug=False,
                   num_devices=NCORES)

    x_idx = nc.dram_tensor("x_idx", [1, 1], I32, kind="ExternalInput")
    emb = nc.dram_tensor("emb", [LEN_ACTION, E], WD, kind="ExternalInput")
    hin_rhs = nc.dram_tensor("hin_rhs", [128, 2 * KH], WD,
                             kind="ExternalInput")
    hp = nc.dram_tensor("hp", [1, 2 * SH], F32, kind="ExternalInput")
    brow = nc.dram_tensor("brow", [1, 2 * 4 * SH], WD, kind="ExternalInput")
    bheadw = nc.dram_tensor("bheadw", [1, NV], WD, kind="ExternalInput")
    wih0T = nc.dram_tensor("wih0T", [E, R], WD, kind="ExternalInput")
    whh0T = nc.dram_tensor("whh0T", [H, R], WD, kind="ExternalInput")
    wih1T = nc.dram_tensor("wih1T", [H, R], WD, kind="ExternalInput")
    whh1T = nc.dram_tensor("whh1T", [H, R], WD, kind="ExternalInput")
    wheadT = nc.dram_tensor("wheadT", [128, KH * NV], WD,
                            kind="ExternalInput")
    probs_o = nc.dram_tensor("probs", [1, NV], F32, kind="ExternalOutput")
    hout_o = nc.dram_tensor("h_out", [2, H], F32, kind="ExternalOutput")

    with tile.TileContext(nc) as tc:
        with (
            tc.tile_pool(name="wp", bufs=1) as wp,
            tc.tile_pool(name="sb", bufs=1) as sb,
            tc.tile_pool(name="ps", bufs=1, space="PSUM") as ps,
            tc.tile_pool(name="dram", bufs=1, space="DRAM") as dram,
        ):
            # ---------- small input loads ----------
            xs = sb.tile([1, 1], I32, tag="xs")
            nc.sync.dma_start(xs[:], x_idx[:, :])
            embsb = sb.tile([LEN_ACTION, E], WD, tag="embsb")
            nc.sync.dma_start(embsb[:], emb[:, :])
            hin_sb = sb.tile([128, 2 * KH], WD, tag="hin_sb")
            nc.sync.dma_start(hin_sb[:], hin_rhs[:, :])
            hp_sb = sb.tile([1, 2 * SH], F32, tag="hp_sb")
            nc.sync.dma_start(hp_sb[:], hp[:, :])
            b_sb = sb.tile([1, 2 * 4 * SH], WD, tag="b_sb")
            nc.sync.dma_start(b_sb[:], brow[:, :])
            whead_sb = sb.tile([128, KH * NV], WD, tag="whead_sb")
            nc.sync.dma_start(whead_sb[:], wheadT[:, :])
            bh_sb = sb.tile([1, NV], WD, tag="bh_sb")
            nc.sync.dma_start(bh_sb[:], bheadw[:, :])
            ones_w = sb.tile([1, 1], WD, tag="ones_w")
            nc.vector.memset(ones_w[:], 1.0)

            # ---------- weight loads: 7 big DMAs ----------
            def wload(wdram, nk, name, halves):
                tiles = []
                per = nk // halves
                for hhf in range(halves):
                    t = wp.tile([128, per * R], WD, tag=f"{name}_{hhf}")
                    src = wdram.ap()[hhf * per * 128:(hhf + 1) * per * 128, :]
                    nc.sync.dma_start(
                        t[:].rearrange("p (kb m) -> p kb m", kb=per),
                        src.rearrange("(kb p) m -> p kb m", p=128))
                    tiles.append(t)

                def slab(k):          # [128, R] k-slab view
                    hhf, kb = divmod(k, per)
                    return tiles[hhf][:, kb * R:(kb + 1) * R]
                return slab

            wih0 = wload(wih0T, KE, "wih0", 1)
            whh0 = wload(whh0T, KH, "whh0", 2)
            whh1 = wload(whh1T, KH, "whh1", 2)
            wih1 = wload(wih1T, KH, "wih1", 2)

            # ---------- e = emb[x] via one-hot matmul ----------
            xf = sb.tile([1, 1], F32, tag="xf")
            nc.vector.tensor_copy(xf[:], xs[:])
            ones = sb.tile([1, 128], F32, tag="ones")
            nc.vector.memset(ones[:], 1.0)
            pse = ps.tile([128, KE + 1], F32, tag="pse")
            nc.tensor.matmul(pse[:, KE:KE + 1], ones[:], xf[:],
                             start=True, stop=False, skip_group_check=True)
            iot = sb.tile([128, 1], F32, tag="iot")
            nc.gpsimd.iota(iot[:], [[0, 1]], channel_multiplier=1,
                           allow_small_or_imprecise_dtypes=True)
            oneh = sb.tile([128, 1], WD, tag="oneh")
            nc.vector.tensor_tensor(oneh[:], iot[:], pse[:, KE:KE + 1],
                                    mybir.AluOpType.is_equal)
            for c in range(KE):
                nc.tensor.matmul(pse[:, c:c + 1],
                                 embsb[0:LEN_ACTION, c * 128:(c + 1) * 128],
                                 oneh[0:LEN_ACTION, 0:1],
                                 start=False, stop=(c == KE - 1),
                                 skip_group_check=True)
            e_sb = sb.tile([128, KE], WD, tag="e_sb")
            nc.vector.tensor_copy(e_sb[:], pse[:, 0:KE])

            # ---------- matvec block: weights stream as rhs ----------
            def mm_block(pA, pBC, slab_fn, nk, x_fn, n_off, firstA, firstBC,
                         lastA, lastBC):
                """psum A [1,512] += slab[:, 0:512].T @ x  (rz, gi+gh fused)
                psum BC[1, n_off:n_off+256] (+)= slab[:, 512:768].T @ x."""
                for k in range(nk):
                    sl = slab_fn(k)
                    xk = x_fn(k)
                    nc.tensor.matmul(pA[0:1, :], xk, sl[:, 0:2 * SH],
                                     start=firstA and k == 0,
                                     stop=lastA and k == nk - 1,
                                     skip_group_check=True)
                    nc.tensor.matmul(pBC[0:1, n_off:n_off + SH], xk,
                                     sl[:, 2 * SH:3 * SH],
                                     start=firstBC and k == 0,
                                     stop=lastBC and k == nk - 1,
                                     skip_group_check=True)

            def bias_mms(pA, pBC, l):
                """Seed each psum bank with its bias row via rank-1
                matmuls (start=True = bank clear), so the weight matmuls
                accumulate on top and the gate math reads biased sums."""
                bo = l * 4 * SH
                nc.tensor.matmul(pA[0:1, :], ones_w[:],
                                 b_sb[0:1, bo:bo + 2 * SH],
                                 start=True, stop=False,
                                 skip_group_check=True)
                nc.tensor.matmul(pBC[0:1, 0:SH], ones_w[:],
                                 b_sb[0:1, bo + 2 * SH:bo + 3 * SH],
                                 start=True, stop=False,
                                 skip_group_check=True)
                nc.tensor.matmul(pBC[0:1, SH:2 * SH], ones_w[:],
                                 b_sb[0:1, bo + 3 * SH:bo + 4 * SH],
                                 start=False, stop=False,
                                 skip_group_check=True)

            def gates(pA, pBC, l):
                """GRU gate math on partition 0 (biases already in psum).
                pA [1,512] = i_rz+h_rz+b_rz; pBC [1,512] = i_n | h_n."""
                rz = sb.tile([1, 2 * SH], F32, tag=f"rz{l}")
                nc.scalar.activation(rz[:], pA[0:1, :],
                                     mybir.ActivationFunctionType.Sigmoid)
                rhn = sb.tile([1, SH], F32, tag=f"rhn{l}")
                nc.vector.tensor_mul(rhn[:], rz[0:1, 0:SH],
                                     pBC[0:1, SH:2 * SH])
                npre = sb.tile([1, SH], F32, tag=f"npre{l}")
                nc.vector.tensor_add(npre[:], pBC[0:1, 0:SH], rhn[:])
                n = sb.tile([1, SH], F32, tag=f"n{l}")
                nc.scalar.activation(n[:], npre[:],
                                     mybir.ActivationFunctionType.Tanh)
                d = sb.tile([1, SH], F32, tag=f"d{l}")
                nc.vector.tensor_sub(d[:], hp_sb[0:1, l * SH:(l + 1) * SH],
                                     n[:])
                zd = sb.tile([1, SH], F32, tag=f"zd{l}")
                nc.vector.tensor_mul(zd[:], rz[0:1, SH:2 * SH], d[:])
                h = sb.tile([1, SH], F32, tag=f"h{l}")
                nc.vector.tensor_add(h[:], n[:], zd[:])
                return h

            def allgather(h_tile, l):
                ag_in = dram.tile([SH], mybir.dt.float32, tag=f"agi{l}")
                ag_out = dram.tile([H], mybir.dt.float32, tag=f"ago{l}")
                nc.sync.dma_start(ag_in[:].unsqueeze(0), h_tile[0:1, :])
                nc.gpsimd.collective_compute(
                    "AllGather",
                    mybir.AluOpType.bypass,
                    replica_groups=[list(range(NCORES))],
                    ins=[ag_in[:].opt()],
                    outs=[ag_out[:].opt()],
                )
                # chunked [128, 16] readback (lhsT x-chunks for next use)
                hf = sb.tile([128, KH], F32, tag=f"hf{l}")
                nc.sync.dma_start(hf[:],
                                  ag_out.rearrange("(c p) -> p c", p=128))
                # full-state output row l
                nc.sync.dma_start(
                    hout_o.ap()[l, :].rearrange("(c p) -> p c", p=128),
                    hf[:])
                if WD is F32:
                    return hf
                hfw = sb.tile([128, KH], WD, tag=f"hfw{l}")
                nc.vector.tensor_copy(hfw[:], hf[:])
                return hfw

            # ---------- layer 0 ----------
            pA0 = ps.tile([1, 2 * SH], F32, tag="pA0")
            pBC0 = ps.tile([1, 2 * SH], F32, tag="pBC0")
            bias_mms(pA0, pBC0, 0)
            mm_block(pA0, pBC0, wih0, KE, lambda k: e_sb[:, k:k + 1],
                     n_off=0, firstA=False, firstBC=False,
                     lastA=False, lastBC=True)
            mm_block(pA0, pBC0, whh0, KH, lambda k: hin_sb[:, k:k + 1],
                     n_off=SH, firstA=False, firstBC=False,
                     lastA=True, lastBC=True)
            h0 = gates(pA0, pBC0, 0)
            h0f = allgather(h0, 0)

            # ---------- layer 1 (gh first: its rhs is known from t=0) ----
            pA1 = ps.tile([1, 2 * SH], F32, tag="pA1")
            pBC1 = ps.tile([1, 2 * SH], F32, tag="pBC1")
            bias_mms(pA1, pBC1, 1)
            mm_block(pA1, pBC1, whh1, KH,
                     lambda k: hin_sb[:, KH + k:KH + k + 1],
                     n_off=SH, firstA=False, firstBC=False,
                     lastA=False, lastBC=True)
            mm_block(pA1, pBC1, wih1, KH, lambda k: h0f[:, k:k + 1],
                     n_off=0, firstA=False, firstBC=False,
                     lastA=True, lastBC=True)
            h1 = gates(pA1, pBC1, 1)
            h1f = allgather(h1, 1)

            # ---------- head + softmax (redundant on every core) --------
            psh = ps.tile([1, NV], F32, tag="psh")
            nc.tensor.matmul(psh[0:1, :], ones_w[:], bh_sb[:],
                             start=True, stop=False, skip_group_check=True)
            for c in range(KH):
                nc.tensor.matmul(psh[0:1, :], h1f[:, c:c + 1],
                                 whead_sb[:, c * NV:(c + 1) * NV],
                                 start=False, stop=(c == KH - 1),
                                 skip_group_check=True)
            # logits are O(1) here, so exp without max-subtraction is safe
            ex = sb.tile([1, NV], F32, tag="ex")
            nc.scalar.activation(ex[:], psh[0:1, :],
                                 mybir.ActivationFunctionType.Exp)
            ssum = sb.tile([1, 1], F32, tag="ssum")
            nc.vector.tensor_reduce(ssum[:], ex[:],
                                    axis=mybir.AxisListType.X,
                                    op=mybir.AluOpType.add)
            rs = sb.tile([1, 1], F32, tag="rs")
            nc.vector.reciprocal(rs[:], ssum[:])
            prb = sb.tile([1, NV], F32, tag="prb")
            nc.vector.tensor_scalar(prb[:], ex[:], rs[0:1, 0:1], None,
                                    mybir.AluOpType.mult)
            nc.sync.dma_start(probs_o[:, :], prb[:])

    nc.compile()
    return nc


def _chunk(v):
    """[n*128] -> [128, n] chunked layout: out[p, c] = v[c*128 + p]."""
    return np.ascontiguousarray(v.reshape(-1, 128).T)


def _prep_inputs(x, h_in, emb, w_ih0, w_hh0, b_ih0, b_hh0,
                 w_ih1, w_hh1, b_ih1, b_hh1, w_head, b_head, np_wdt):
    """Build the per-core input maps (shard + transpose + cast)."""
    f32 = np.float32
    x_i = np.asarray(x).astype(np.int32).reshape(1, 1)
    h_in = np.asarray(h_in, f32)
    emb = np.ascontiguousarray(np.asarray(emb)).astype(np_wdt)
    w = {0: (np.asarray(w_ih0, f32), np.asarray(w_hh0, f32)),
         1: (np.asarray(w_ih1, f32), np.asarray(w_hh1, f32))}
    b = {0: (np.asarray(b_ih0, f32), np.asarray(b_hh0, f32)),
         1: (np.asarray(b_ih1, f32), np.asarray(b_hh1, f32))}
    w_head = np.asarray(w_head, f32)
    b_head = np.asarray(b_head, f32).reshape(1, NV)

    hin_rhs = np.concatenate([_chunk(h_in[0, 0]), _chunk(h_in[1, 0])],
                             axis=1).astype(np_wdt)
    # wheadT[p, c*16+j] = w_head[j, c*128+p]
    wheadT = np.ascontiguousarray(
        w_head.T.reshape(KH, 128, NV).transpose(1, 0, 2).reshape(128, KH * NV)
    ).astype(np_wdt)

    in_maps = []
    for k in range(NCORES):
        sl = slice(k * SH, (k + 1) * SH)

        def rows(mat):
            return np.concatenate([mat[g * H:(g + 1) * H][sl]
                                   for g in range(3)], axis=0)

        core = {
            "x_idx": x_i, "emb": emb, "hin_rhs": hin_rhs, "wheadT": wheadT,
            "bheadw": b_head.astype(np_wdt),
            "hp": np.concatenate([h_in[0, 0, sl],
                                  h_in[1, 0, sl]]).reshape(1, 2 * SH),
        }
        for l in (0, 1):
            wih, whh = w[l]
            core[f"wih{l}T"] = np.ascontiguousarray(rows(wih).T).astype(np_wdt)
            core[f"whh{l}T"] = np.ascontiguousarray(rows(whh).T).astype(np_wdt)
        blobs = []
        for l in (0, 1):
            bih, bhh = b[l]
            brz = (bih + bhh)
            blobs += [np.concatenate([brz[g * H:(g + 1) * H][sl]
                                      for g in range(2)]),
                      bih[2 * H:][sl], bhh[2 * H:][sl]]
        core["brow"] = np.concatenate(blobs).reshape(1, -1).astype(np_wdt)
        in_maps.append(core)
    return in_maps


def kernel(x, h_in, emb, w_ih0, w_hh0, b_ih0, b_hh0,
           w_ih1, w_hh1, b_ih1, b_hh1, w_head, b_head):
    global LAST_RESULTS
    import ml_dtypes
    from concourse.bass_utils import run_bass_kernel_spmd

    wdt = WDT
    np_wdt = np.float32 if wdt == "f32" else ml_dtypes.bfloat16
    trace = bool(int(os.environ.get("BASS_KERNEL_TRACE", "0")))

    key = (wdt,)
    if key not in _CACHE:
        _CACHE[key] = _build(wdt)
    nc = _CACHE[key]

    in_maps = _prep_inputs(x, h_in, emb, w_ih0, w_hh0, b_ih0, b_hh0,
                           w_ih1, w_hh1, b_ih1, b_hh1, w_head, b_head,
                           np_wdt)
    res = run_bass_kernel_spmd(nc, in_maps, core_ids=list(range(NCORES)),
                               trace=trace)
    LAST_RESULTS = res
    probs = np.asarray(res.results[0]["probs"], np.float32)
    h_out = np.asarray(res.results[0]["h_out"],
                       np.float32).reshape(2, 1, H)
    return probs, h_out
